# revision 4
# baseline (speedup 1.0000x reference)
"""GRU decoder (nn_Decoder2) Trainium2 Bass kernel, v3.

Per core (pure batch data-parallel over 8 cores): 4096 rows, 8 chunks of 512,
2 supergroups (SG) of 4 chunks, partition-stacked [128, 512] so elementwise
runs 128 lanes wide.  v2 was ~290us; v3 restructures to cut PE streams and
the serial chain:

  - PE streams per SG per step: 7 (was 8+ident bookkeeping):
      z: feat-mm (K=64) + h-mm (K=128, rk+dw*k0 fold)
      r: feat-mm (K=64) + h-mm (K=128, fold)
      m: h-mm only (K=128, pure rk; bias via stt scalar)
      x: feat-mm (K=68, rows 64:67 = s-rows carrying y-feedback) + t2-inject
    The v2 whh_x matmul (rank-1 dw x k0 dense-fold) is replaced by 4 s-rows
    in the x feat-mm rhs: s(t) = h(t) @ dw computed by a per-step dense mm
    (tile_position (0,64) parks it at partitions 64:68 of the just-consumed
    X psum region), evacuated fp16 into the NEXT step's feat-tile s-rows.
    The same s-rows are the output DMA source (y = s + dense_b host-side).
  - Biases moved off the feat-mms into ACT per-partition bias APs:
    sigmoid(z/r + bias_zr[128,1]), tanh(x + bias_x[128,1]).  Feat lhsT has
    no ones row; no special t=0 feat weights (t=0 s-rows = y0 - db, host).
  - Blend reform: h' = z*h + (1-z)*hh with omz = 1-z (tensor_scalar) and
    zh = z*h precomputed on GPSIMD while tanh runs; after tanh only
    c = omz*hh (DVE) + h' = zh + c (DVE) remain on the serial chain.
  - t=0 keeps wh0_z/wh0_r (pure rk, y0 enters via s-rows).

Weights prepared in float64, quantized fp16 (PSUM accumulates fp32).
"""
import numpy as np

B, T, F, H = 32768, 48, 16, 32
NCORES = 8
BS = B // NCORES            # 4096 batch per core
CK = 512                    # chunk batch size
NSG = 2                     # supergroups
SGC = 4                     # chunks per supergroup

_CACHE = {}


def _prep_weights(kernel, recurrent_kernel, bias_x, bias_h, dense_w, dense_b):
    """Build v3 weight tiles in float64, return fp32 dict."""
    kd = kernel.astype(np.float64)
    rkd = recurrent_kernel.astype(np.float64)
    bxd = bias_x.astype(np.float64)
    bhd = bias_h.astype(np.float64)
    dwd = dense_w.astype(np.float64)[:, 0]          # [32]
    dbd = float(dense_b.astype(np.float64)[0])

    k0 = kd[0]                                      # [96]
    kf = kd[1:]                                     # [16, 96]
    dwk0 = np.outer(dwd, k0)                        # [32, 96]

    out = {}
    blocks = {"z": slice(0, 32), "r": slice(32, 64), "x": slice(64, 96)}
    # biases: z/r get bx+bh (reset_after applies bh_z/bh_r outside the gate
    # matmuls identically), x gets bx only (bh_m rides the stt scalar).
    bias1 = {"z": bxd[0:32] + bhd[0:32], "r": bxd[32:64] + bhd[32:64],
             "x": bxd[64:96]}
    for g, blk in blocks.items():
        # feat weights [68, 128]: rows 0:64 block-diag kf, rows 64:68 =
        # per-chunk k0 (s-row weights; z/r only use them at t=0, x always).
        wfr = np.zeros((68, 128), np.float64)
        for c in range(4):
            cols = slice(32 * c, 32 * c + 32)
            wfr[16 * c:16 * c + 16, cols] = kf[:, blk]
            wfr[64 + c, cols] = k0[blk]
        out[f"wf_{g}"] = wfr
        # per-partition bias AP [128,1]: bias1 + db*k0 (s = y - db).
        out[f"bias_{g}"] = np.tile(bias1[g] + dbd * k0[blk], 4).reshape(128, 1)
        if g != "x":
            whh = np.zeros((128, 128), np.float64)
            wh0 = np.zeros((128, 128), np.float64)
            for c in range(4):
                rows = slice(32 * c, 32 * c + 32)
                cols = slice(32 * c, 32 * c + 32)
                whh[rows, cols] = rkd[:, blk] + dwk0[:, blk]
                wh0[rows, cols] = rkd[:, blk]
            out[f"whh_{g}"] = whh
            out[f"wh0_{g}"] = wh0
    whm = np.zeros((128, 128), np.float64)          # mh: h-only, all t
    for c in range(4):
        whm[32 * c:32 * c + 32, 32 * c:32 * c + 32] = rkd[:, 64:96]
    out["whh_m"] = whm
    wd4 = np.zeros((128, 4), np.float64)
    for c in range(4):
        wd4[32 * c:32 * c + 32, c] = dwd
    out["wd4"] = wd4
    out["bhm"] = np.tile(bhd[64:96], 4).reshape(128, 1)
    out["ident"] = np.eye(128)
    return {k: np.ascontiguousarray(v.astype(np.float32)) for k, v in out.items()}


_F32W = ("bhm", "bias_z", "bias_r", "bias_x")


def _build_module(n_steps=T):
    import concourse.bacc as bacc
    import concourse.mybir as mybir
    import concourse.tile as tile
    from contextlib import ExitStack

    f32 = mybir.dt.float32
    f16 = mybir.dt.float16
    AF = mybir.ActivationFunctionType
    ALU = mybir.AluOpType

    nc = bacc.Bacc("TRN2")
    # feat_packed [T, 68, 1024] fp16: rows 0:64 = chunk-stacked feats,
    # rows 64:68 = per-chunk s-rows (t=0: y0-db from host; t>0: evac'd).
    feat = nc.dram_tensor("feat", [n_steps, 68, NSG * CK], f16,
                          kind="ExternalInput")
    h0 = nc.dram_tensor("h0", [H, BS], f16, kind="ExternalInput")
    wnames_f = ["wf_z", "wf_r", "wf_x"]
    wnames_h = ["whh_z", "whh_r", "whh_m"]
    wnames_h0 = ["wh0_z", "wh0_r", "whh_m"]
    dram_w = {}
    for n in wnames_f:
        dram_w[n] = nc.dram_tensor(n, [68, 128], f16, kind="ExternalInput")
    for n in set(wnames_h + wnames_h0):
        dram_w[n] = nc.dram_tensor(n, [128, 128], f16, kind="ExternalInput")
    dram_w["wd4"] = nc.dram_tensor("wd4", [128, 4], f16, kind="ExternalInput")
    dram_w["ident"] = nc.dram_tensor("ident", [128, 128], f16,
                                     kind="ExternalInput")
    for n in _F32W:
        dram_w[n] = nc.dram_tensor(n, [128, 1], f32, kind="ExternalInput")
    out = nc.dram_tensor("out", [n_steps, BS], f16, kind="ExternalOutput")

    NBLK = (n_steps + 3) // 4

    with tile.TileContext(nc) as tc, ExitStack() as ctx:
        wpool = ctx.enter_context(tc.tile_pool(name="weights", bufs=1))
        xpool = ctx.enter_context(tc.tile_pool(name="featp", bufs=4))
        hpool = ctx.enter_context(tc.tile_pool(name="hs", bufs=1))
        ew = ctx.enter_context(tc.tile_pool(name="ew", bufs=4))
        dpool = ctx.enter_context(tc.tile_pool(name="dsb", bufs=2))
        ppool = ctx.enter_context(tc.tile_pool(name="psum", bufs=1,
                                               space="PSUM"))

        ws = {}
        for n, d in dram_w.items():
            dt = f32 if n in _F32W else f16
            t_ = wpool.tile(list(d.shape), dt, tag=n)
            nc.sync.dma_start(t_[:, :], d[:, :])
            ws[n] = t_

        # 5x-buffered stacked state tiles per SG [128, 512] fp16:
        # blend(t) writes buf[t%5]; h-mms(t) read buf[(t+4)%5].
        h_s = [[hpool.tile([128, CK], f16, tag=f"hs{g}_{p}", name=f"hs{g}_{p}")
                for p in range(5)] for g in range(NSG)]
        for g in range(NSG):
            for ci in range(SGC):
                c = g * SGC + ci
                nc.sync.dma_start(h_s[g][4][32 * ci:32 * ci + 32, :],
                                  h0[:, c * CK:(c + 1) * CK])

        # Feat block tiles [68, 2048] per SG (block b = steps 4b..4b+3).
        feat_t = [[None] * NBLK for _ in range(NSG)]

        def load_block(b):
            t0 = 4 * b
            ns = min(t0 + 4, n_steps) - t0
            nr = 68 if b == 0 else 64   # s-rows only come from HBM at t=0
            for g in range(NSG):
                t_ = xpool.tile([68, 4 * CK], f16, tag=f"feat{g}",
                                name=f"feat{g}_b{b}")
                nc.sync.dma_start(
                    t_[0:nr, 0:ns * CK].rearrange("r (s b) -> r s b", s=ns),
                    feat[t0:t0 + ns, 0:nr, g * CK:(g + 1) * CK].rearrange(
                        "s r b -> r s b"))
                feat_t[g][b] = t_

        load_block(0)
        if NBLK > 1:
            load_block(1)

        # tail s-tile for the last step's output
        tail_s = [dpool.tile([4, CK], f16, tag=f"tail{g}", name=f"tail{g}")
                  for g in range(NSG)]

        zr_ps = [ppool.tile([128, 2 * CK], f32, tag=f"zr{g}", name=f"zr{g}")
                 for g in range(NSG)]
        xm_ps = [ppool.tile([128, 2 * CK], f32, tag=f"xm{g}", name=f"xm{g}")
                 for g in range(NSG)]

        def bank(g, which):
            if which == "z":
                return zr_ps[g][:, 0:CK]
            if which == "r":
                return zr_ps[g][:, CK:2 * CK]
            if which == "x":
                return xm_ps[g][:, 0:CK]
            return xm_ps[g][:, CK:2 * CK]   # m

        def emit_feat(tt):
            """Feat-mms for step tt: z/r use K=64 (no s-row dependency,
            except t=0 where y0 enters all gates), x uses K=68 (s-rows)."""
            bb, ss = tt // 4, tt % 4
            kzr = 68 if tt == 0 else 64
            for gate, kk in (("r", kzr), ("z", kzr), ("x", 68)):
                wn = f"wf_{gate}"
                for g in range(NSG):
                    rhs = feat_t[g][bb][0:kk, ss * CK:(ss + 1) * CK]
                    nc.tensor.matmul(bank(g, gate),
                                     lhsT=ws[wn][0:kk, :], rhs=rhs,
                                     start=True, stop=False,
                                     tile_position=(0, 0))

        for t in range(n_steps):
            blk, slot = t // 4, t % 4
            hb_in = [h_s[g][(t + 4) % 5] for g in range(NSG)]

            if t == 0:
                emit_feat(0)

            # -- PE: h-mms, gate-paired across SGs; r first ------------
            for gate in ("r", "z", "m"):
                wn = "whh_m" if gate == "m" else (
                    f"wh0_{gate}" if t == 0 else f"whh_{gate}")
                for g in range(NSG):
                    nc.tensor.matmul(bank(g, gate), lhsT=ws[wn][:, :],
                                     rhs=hb_in[g][:, :],
                                     start=(gate == "m"), stop=True,
                                     tile_position=(0, 0))

            # -- elementwise per SG, op-major emission ----------------
            ewt = {}
            for g in range(NSG):
                ewt[g] = {
                    "rs": ew.tile([128, CK], f16, tag=f"rs{g}", name=f"rs{g}_{t}"),
                    "zs": ew.tile([128, CK], f16, tag=f"zs{g}", name=f"zs{g}_{t}"),
                    "t2": ew.tile([128, CK], f16, tag=f"t2{g}", name=f"t2{g}_{t}"),
                    "hh": ew.tile([128, CK], f16, tag=f"hh{g}", name=f"hh{g}_{t}"),
                    "omz": ew.tile([128, CK], f16, tag=f"omz{g}", name=f"omz{g}_{t}"),
                    "zh": ew.tile([128, CK], f16, tag=f"zh{g}", name=f"zh{g}_{t}"),
                    "c": ew.tile([128, CK], f16, tag=f"c{g}", name=f"c{g}_{t}"),
                }
            for g in range(NSG):
                nc.scalar.activation(ewt[g]["rs"][:, :], bank(g, "r"),
                                     AF.Sigmoid, bias=ws["bias_r"][:, 0:1])
            for g in range(NSG):
                nc.vector.scalar_tensor_tensor(
                    ewt[g]["t2"][:, :], bank(g, "m"), ws["bhm"][:, 0:1],
                    ewt[g]["rs"][:, :], ALU.add, ALU.mult)
            for g in range(NSG):
                nc.scalar.activation(ewt[g]["zs"][:, :], bank(g, "z"),
                                     AF.Sigmoid, bias=ws["bias_z"][:, 0:1])
            for g in range(NSG):
                # t3: accumulate t2 into the x psum bank (closes the group)
                nc.tensor.matmul(bank(g, "x"), lhsT=ws["ident"][:, :],
                                 rhs=ewt[g]["t2"][:, :], start=False,
                                 stop=True, tile_position=(0, 0))
            # off-chain blend prep on GPSIMD while tanh runs
            for g in range(NSG):
                nc.gpsimd.tensor_scalar(ewt[g]["omz"][:, :], ewt[g]["zs"][:, :],
                                        -1.0, 1.0, ALU.mult, ALU.add)
            for g in range(NSG):
                nc.scalar.activation(ewt[g]["hh"][:, :], bank(g, "x"),
                                     AF.Tanh, bias=ws["bias_x"][:, 0:1])
            for g in range(NSG):
                nc.gpsimd.tensor_mul(ewt[g]["zh"][:, :], ewt[g]["zs"][:, :],
                                     hb_in[g][:, :])
            for g in range(NSG):
                nc.vector.tensor_mul(ewt[g]["c"][:, :], ewt[g]["omz"][:, :],
                                     ewt[g]["hh"][:, :])
            for g in range(NSG):
                nc.vector.tensor_add(h_s[g][t % 5][:, :], ewt[g]["zh"][:, :],
                                     ewt[g]["c"][:, :])

            # -- dense s(t) = h(t) @ dw, parked at partitions 64:68 of the
            #    consumed X region; evac into next step's s-rows + out DMA.
            for g in range(NSG):
                nc.tensor.matmul(xm_ps[g][64:68, 0:CK], lhsT=ws["wd4"][:, :],
                                 rhs=h_s[g][t % 5][:, :], start=True,
                                 stop=True, tile_position=(0, 64))
            for g in range(NSG):
                if t + 1 < n_steps:
                    nb, nslot = (t + 1) // 4, (t + 1) % 4
                    sdst = feat_t[g][nb][64:68, nslot * CK:(nslot + 1) * CK]
                else:
                    sdst = tail_s[g][:, :]
                nc.vector.tensor_copy(sdst, xm_ps[g][64:68, 0:CK])
                gb = g * SGC * CK
                nc.sync.dma_start(
                    out[t, gb:gb + 4 * CK].rearrange("(c b) -> c b", c=4),
                    sdst)

            # -- PE: feat-mms for next step ----------------------------
            if t + 1 < n_steps:
                emit_feat(t + 1)

            # -- prefetch feat block -----------------------------------
            if slot == 3 and blk + 2 < NBLK:
                load_block(blk + 2)

    nc.compile()
    return nc


def _host_prep(inputs, n_steps=T):
    """Shard + pack inputs host-side. Returns (in_maps, dense_b)."""
    dfeat = np.asarray(inputs["decoder_feature"], np.float32)
    y0 = np.asarray(inputs["decoder_init_input"], np.float32)
    h0 = np.asarray(inputs["init_state"], np.float32)
    ws = _prep_weights(
        np.asarray(inputs["kernel"], np.float32),
        np.asarray(inputs["recurrent_kernel"], np.float32),
        np.asarray(inputs["bias_x"], np.float32),
        np.asarray(inputs["bias_h"], np.float32),
        np.asarray(inputs["dense_w"], np.float32),
        np.asarray(inputs["dense_b"], np.float32),
    )
    wmap = {k: v.astype(np.float32 if k in _F32W else np.float16)
            for k, v in ws.items()}

    db = float(np.asarray(inputs["dense_b"], np.float64)[0])

    def one(sl):
        fx = np.zeros((n_steps, 68, NSG * CK), np.float32)
        dv = dfeat[sl, :n_steps]                     # [BS, T, F]
        dv = dv.reshape(NSG, SGC, CK, n_steps, F)
        fx[:, 0:64, :] = (dv.transpose(3, 1, 4, 0, 2)   # [T,SGC,F,NSG,CK]
                          .reshape(n_steps, 64, NSG * CK))
        yv = y0[sl, 0].reshape(NSG, SGC, CK)             # [g, ci, b]
        fx[0, 64:68, :] = (yv.transpose(1, 0, 2)
                           .reshape(4, NSG * CK) - db)
        m = {
            "feat": np.ascontiguousarray(fx).astype(np.float16),
            "h0": np.ascontiguousarray(h0[sl].T).astype(np.float16),
        }
        m.update(wmap)
        return m

    in_maps = [one(slice(i * BS, (i + 1) * BS)) for i in range(NCORES)]
    return in_maps, db


def run(inputs, trace=False, n_steps=T, **spmd_kwargs):
    """Run on the 8 NeuronCores; returns (out [B,T,1] fp32, results)."""
    from concourse.bass_utils import run_bass_kernel_spmd

    key = n_steps
    if key not in _CACHE:
        _CACHE[key] = _build_module(n_steps)
    nc = _CACHE[key]
    in_maps, db = _host_prep(inputs, n_steps)
    res = run_bass_kernel_spmd(nc, in_maps, list(range(NCORES)),
                               trace=trace, **spmd_kwargs)
    outs = np.concatenate(
        [np.asarray(r["out"]).astype(np.float32) for r in res.results], axis=1)
    full = (outs.T[:, :, None] + np.float32(db)).astype(np.float32)
    return full, res


def kernel(**inputs) -> np.ndarray:
    out, _ = run(inputs, trace=False)
    return out


# revision 5
# speedup vs baseline: 1.2965x; 1.2965x over previous
"""GRU decoder (nn_Decoder2) Trainium2 Bass kernel, v3.1.

Per core (pure batch data-parallel over 8 cores): 4096 rows, 8 chunks of 512,
2 supergroups (SG) of 4 chunks, partition-stacked [128, 512] so elementwise
runs 128 lanes wide.  Structure follows v2 (~290us) with three changes aimed
at PE head-of-line stalls and serial-chain length:

  - PE emission order per step: h-mms (r,z,m,x) -> sig r -> stt -> sig z ->
    r/z feat-mms for t+1 (only WAR on the sigmoids, so the PE FIFO isn't
    blocked behind the t2-inject while ready work exists) -> dense parks
    (fill the tanh wait window on t%4==0 steps) -> inject -> tanh ->
    x-feat-mm for t+1.  v2 queued all feat(t+1) behind the inject, idling
    the PE ~0.7us/step at the stt wait.
  - Biases moved off the feat-mms into ACT per-partition bias APs:
    sigmoid(z/r + bias_zr[128,1]), tanh(x + bias_x[128,1]); feat lhsT loses
    its ones row (K=64 for t>=1; K=68 at t=0 where rows 64:68 = y0-db
    per-chunk carry the initial y feedback for all gates).
  - Blend reform: h' = z*h + (1-z)*hh with omz = 1-z on GPSIMD (off-chain,
    overlaps tanh) and zh = z*h on DVE before tanh completes; after tanh
    only c = omz*hh and h' = zh + c remain on the serial chain (v2 had
    sub -> mul -> add, one more chain link).

Dense(1) -> next-input dependency folded into the recurrent weights
(whh_z/r = rk + dw x k0, whh_x = dw x k0, bias += db*k0), so the recurrence
never waits on the dense output.  Dense head batched per 4 steps: 8 mms park
at partition offsets 32*tau of the freed Z psum region, one DVE tensor_copy
evacuates all parks, 4 out-DMAs per SG per block.  Weights are prepared in
float64 and quantized to fp16 (PSUM accumulates fp32).
"""
import numpy as np

B, T, F, H = 32768, 48, 16, 32
NCORES = 8
BS = B // NCORES            # 4096 batch per core
CK = 512                    # chunk batch size
NSG = 2                     # supergroups
SGC = 4                     # chunks per supergroup

_CACHE = {}


def _prep_weights(kernel, recurrent_kernel, bias_x, bias_h, dense_w, dense_b):
    """Build v3.1 weight tiles in float64, return fp32 dict."""
    kd = kernel.astype(np.float64)
    rkd = recurrent_kernel.astype(np.float64)
    bxd = bias_x.astype(np.float64)
    bhd = bias_h.astype(np.float64)
    dwd = dense_w.astype(np.float64)[:, 0]          # [32]
    dbd = float(dense_b.astype(np.float64)[0])

    k0 = kd[0]                                      # [96]
    kf = kd[1:]                                     # [16, 96]
    dwk0 = np.outer(dwd, k0)                        # [32, 96]

    out = {}
    blocks = {"z": slice(0, 32), "r": slice(32, 64), "x": slice(64, 96)}
    bias1 = {"z": bxd[0:32] + bhd[0:32], "r": bxd[32:64] + bhd[32:64],
             "x": bxd[64:96]}
    for g, blk in blocks.items():
        # feat weights [68, 128]: rows 0:64 block-diag kf, rows 64:68 =
        # per-chunk k0 (y0-row weights, only streamed at t=0).
        wfr = np.zeros((68, 128), np.float64)
        whh = np.zeros((128, 128), np.float64)
        for c in range(4):
            cols = slice(32 * c, 32 * c + 32)
            wfr[16 * c:16 * c + 16, cols] = kf[:, blk]
            wfr[64 + c, cols] = k0[blk]
            rows = slice(32 * c, 32 * c + 32)
            if g == "x":
                whh[rows, cols] = dwk0[:, blk]
            else:
                whh[rows, cols] = rkd[:, blk] + dwk0[:, blk]
        out[f"wf_{g}"] = wfr
        out[f"whh_{g}"] = whh
        # per-partition bias AP [128,1]: bias1 + db*k0 (y0-rows carry y0-db).
        out[f"bias_{g}"] = np.tile(bias1[g] + dbd * k0[blk], 4).reshape(128, 1)
        if g != "x":
            wh0 = np.zeros((128, 128), np.float64)
            for c in range(4):
                wh0[32 * c:32 * c + 32, 32 * c:32 * c + 32] = rkd[:, blk]
            out[f"wh0_{g}"] = wh0
    whm = np.zeros((128, 128), np.float64)          # mh: h-only, all t
    for c in range(4):
        whm[32 * c:32 * c + 32, 32 * c:32 * c + 32] = rkd[:, 64:96]
    out["whh_m"] = whm
    wd4 = np.zeros((128, 4), np.float64)
    for c in range(4):
        wd4[32 * c:32 * c + 32, c] = dwd
    out["wd4"] = wd4
    out["bhm"] = np.tile(bhd[64:96], 4).reshape(128, 1)
    out["ident"] = np.eye(128)
    return {k: np.ascontiguousarray(v.astype(np.float32)) for k, v in out.items()}


_F32W = ("bhm", "bias_z", "bias_r", "bias_x")


def _build_module(n_steps=T):
    import concourse.bacc as bacc
    import concourse.mybir as mybir
    import concourse.tile as tile
    from contextlib import ExitStack

    f32 = mybir.dt.float32
    f16 = mybir.dt.float16
    AF = mybir.ActivationFunctionType
    ALU = mybir.AluOpType

    nc = bacc.Bacc("TRN2")
    # feat_packed [T, 68, 1024] fp16: rows 0:64 = chunk-stacked feats,
    # rows 64:68 = per-chunk (y0 - dense_b) at t=0, unused later.
    feat = nc.dram_tensor("feat", [n_steps, 68, NSG * CK], f16,
                          kind="ExternalInput")
    h0 = nc.dram_tensor("h0", [H, BS], f16, kind="ExternalInput")
    dram_w = {}
    for n in ("wf_z", "wf_r", "wf_x"):
        dram_w[n] = nc.dram_tensor(n, [68, 128], f16, kind="ExternalInput")
    for n in ("whh_z", "whh_r", "whh_x", "whh_m", "wh0_z", "wh0_r", "ident"):
        dram_w[n] = nc.dram_tensor(n, [128, 128], f16, kind="ExternalInput")
    dram_w["wd4"] = nc.dram_tensor("wd4", [128, 4], f16, kind="ExternalInput")
    for n in _F32W:
        dram_w[n] = nc.dram_tensor(n, [128, 1], f32, kind="ExternalInput")
    out = nc.dram_tensor("out", [n_steps, BS], f32, kind="ExternalOutput")

    NBLK = (n_steps + 3) // 4

    with tile.TileContext(nc) as tc, ExitStack() as ctx:
        wpool = ctx.enter_context(tc.tile_pool(name="weights", bufs=1))
        xpool = ctx.enter_context(tc.tile_pool(name="featp", bufs=4))
        hpool = ctx.enter_context(tc.tile_pool(name="hs", bufs=1))
        ew = ctx.enter_context(tc.tile_pool(name="ew", bufs=4))
        dpool = ctx.enter_context(tc.tile_pool(name="dsb", bufs=3))
        ppool = ctx.enter_context(tc.tile_pool(name="psum", bufs=1,
                                               space="PSUM"))

        ws = {}
        for n, d in dram_w.items():
            dt = f32 if n in _F32W else f16
            t_ = wpool.tile(list(d.shape), dt, tag=n)
            nc.sync.dma_start(t_[:, :], d[:, :])
            ws[n] = t_

        # 5x-buffered stacked state tiles per SG [128, 512] fp16:
        # blend(t) writes buf[t%5]; h-mms(t) read buf[(t+4)%5]; the dense
        # head reads states up to 4 steps late. h0 preloaded to buf 4.
        h_s = [[hpool.tile([128, CK], f16, tag=f"hs{g}_{p}", name=f"hs{g}_{p}")
                for p in range(5)] for g in range(NSG)]
        for g in range(NSG):
            for ci in range(SGC):
                c = g * SGC + ci
                nc.sync.dma_start(h_s[g][4][32 * ci:32 * ci + 32, :],
                                  h0[:, c * CK:(c + 1) * CK])

        # Feat block tiles [68, 2048] per SG (block b = steps 4b..4b+3).
        feat_t = [[None] * NBLK for _ in range(NSG)]

        def load_block(b):
            t0 = 4 * b
            ns = min(t0 + 4, n_steps) - t0
            nr = 68 if b == 0 else 64   # y0-rows only exist at t=0
            for g in range(NSG):
                t_ = xpool.tile([68, 4 * CK], f16, tag=f"feat{g}",
                                name=f"feat{g}_b{b}")
                nc.sync.dma_start(
                    t_[0:nr, 0:ns * CK].rearrange("r (s b) -> r s b", s=ns),
                    feat[t0:t0 + ns, 0:nr, g * CK:(g + 1) * CK].rearrange(
                        "s r b -> r s b"))
                feat_t[g][b] = t_

        load_block(0)
        if NBLK > 1:
            load_block(1)

        zr_ps = [ppool.tile([128, 2 * CK], f32, tag=f"zr{g}", name=f"zr{g}")
                 for g in range(NSG)]
        xm_ps = [ppool.tile([128, 2 * CK], f32, tag=f"xm{g}", name=f"xm{g}")
                 for g in range(NSG)]

        def bank(g, which):
            if which == "z":
                return zr_ps[g][:, 0:CK]
            if which == "r":
                return zr_ps[g][:, CK:2 * CK]
            if which == "x":
                return xm_ps[g][:, 0:CK]
            return xm_ps[g][:, CK:2 * CK]   # m

        def emit_feat(tt, gates):
            bb, ss = tt // 4, tt % 4
            kk = 68 if tt == 0 else 64
            for gate in gates:
                for g in range(NSG):
                    rhs = feat_t[g][bb][0:kk, ss * CK:(ss + 1) * CK]
                    nc.tensor.matmul(bank(g, gate),
                                     lhsT=ws[f"wf_{gate}"][0:kk, :], rhs=rhs,
                                     start=True, stop=False,
                                     tile_position=(0, 0))

        def emit_dense_block(t0, nt):
            """Dense mms for steps t0..t0+nt-1, parked at partition offset
            32*tau of the Z psum region, one DVE evac, per-step out-DMAs.
            Emitted during step t0+4 after sig z (WAR) and before the
            z-feat wipe for t0+5."""
            for g in range(NSG):
                for tau in range(nt):
                    p0 = 32 * tau
                    nc.tensor.matmul(zr_ps[g][p0:p0 + 4, 0:CK],
                                     lhsT=ws["wd4"][:, :],
                                     rhs=h_s[g][(t0 + tau) % 5][:, :],
                                     start=True, stop=True,
                                     tile_position=(0, p0))
            for g in range(NSG):
                npp = 32 * (nt - 1) + 4
                dsb = dpool.tile([100, CK], f32, tag=f"dsb{g}",
                                 name=f"dsb{g}_{t0}")
                nc.vector.tensor_copy(dsb[0:npp, :], zr_ps[g][0:npp, 0:CK])
                gb = g * SGC * CK
                for tau in range(nt):
                    nc.sync.dma_start(
                        out[t0 + tau, gb:gb + 4 * CK].rearrange(
                            "(c b) -> c b", c=4),
                        dsb[32 * tau:32 * tau + 4, :])

        for t in range(n_steps):
            blk, slot = t // 4, t % 4
            hb_in = [h_s[g][(t + 4) % 5] for g in range(NSG)]

            if t == 0:
                emit_feat(0, ("r", "z", "x"))

            # -- PE: h-mms, gate-paired across SGs; r first ------------
            for gate in ("r", "z", "m", "x"):
                if t == 0 and gate == "x":
                    continue            # x(0) has no h term
                wn = ("whh_m" if gate == "m" else
                      f"wh0_{gate}" if t == 0 and gate != "x" else
                      f"whh_{gate}")
                for g in range(NSG):
                    nc.tensor.matmul(bank(g, gate), lhsT=ws[wn][:, :],
                                     rhs=hb_in[g][:, :],
                                     start=(gate == "m"),
                                     stop=(gate in ("r", "z", "m")),
                                     tile_position=(0, 0))

            ewt = {}
            for g in range(NSG):
                ewt[g] = {
                    k: ew.tile([128, CK], f16, tag=f"{k}{g}",
                               name=f"{k}{g}_{t}")
                    for k in ("rs", "zs", "t2", "hh", "omz", "zh", "c")
                }
            # -- serial chain: sig r -> t2 -----------------------------
            for g in range(NSG):
                nc.scalar.activation(ewt[g]["rs"][:, :], bank(g, "r"),
                                     AF.Sigmoid, bias=ws["bias_r"][:, 0:1])
            for g in range(NSG):
                nc.vector.scalar_tensor_tensor(
                    ewt[g]["t2"][:, :], bank(g, "m"), ws["bhm"][:, 0:1],
                    ewt[g]["rs"][:, :], ALU.add, ALU.mult)
            for g in range(NSG):
                nc.scalar.activation(ewt[g]["zs"][:, :], bank(g, "z"),
                                     AF.Sigmoid, bias=ws["bias_z"][:, 0:1])

            # -- PE: r/z feat-mms for t+1 (ready once sigmoids read) ---
            if t + 1 < n_steps:
                emit_feat(t + 1, ("r",))
            # dense block parks into the Z region right after sig z
            if t > 0 and t % 4 == 0:
                emit_dense_block(t - 4, 4)
            if t + 1 < n_steps:
                emit_feat(t + 1, ("z",))

            # -- PE: t2-inject closes the x bank; tanh reads psum ------
            for g in range(NSG):
                nc.tensor.matmul(bank(g, "x"), lhsT=ws["ident"][:, :],
                                 rhs=ewt[g]["t2"][:, :], start=False,
                                 stop=True, tile_position=(0, 0))
            # off-chain blend prep while inject/tanh run
            for g in range(NSG):
                nc.gpsimd.tensor_scalar(ewt[g]["omz"][:, :],
                                        ewt[g]["zs"][:, :],
                                        -1.0, 1.0, ALU.mult, ALU.add)
            for g in range(NSG):
                nc.vector.tensor_mul(ewt[g]["zh"][:, :], ewt[g]["zs"][:, :],
                                     hb_in[g][:, :])
            for g in range(NSG):
                nc.scalar.activation(ewt[g]["hh"][:, :], bank(g, "x"),
                                     AF.Tanh, bias=ws["bias_x"][:, 0:1])

            # -- PE: x feat-mm for t+1 (WAR behind tanh) ---------------
            if t + 1 < n_steps:
                emit_feat(t + 1, ("x",))

            # -- chain tail: c = omz*hh, h' = zh + c -------------------
            for g in range(NSG):
                nc.vector.tensor_mul(ewt[g]["c"][:, :], ewt[g]["omz"][:, :],
                                     ewt[g]["hh"][:, :])
            for g in range(NSG):
                nc.vector.tensor_add(h_s[g][t % 5][:, :], ewt[g]["zh"][:, :],
                                     ewt[g]["c"][:, :])

            # -- prefetch feat block -----------------------------------
            if slot == 3 and blk + 2 < NBLK:
                load_block(blk + 2)

        last0 = (n_steps - 1) // 4 * 4
        emit_dense_block(last0, n_steps - last0)
    nc.compile()
    return nc


def _host_prep(inputs, n_steps=T):
    """Shard + pack inputs host-side. Returns (in_maps, dense_b)."""
    dfeat = np.asarray(inputs["decoder_feature"], np.float32)
    y0 = np.asarray(inputs["decoder_init_input"], np.float32)
    h0 = np.asarray(inputs["init_state"], np.float32)
    ws = _prep_weights(
        np.asarray(inputs["kernel"], np.float32),
        np.asarray(inputs["recurrent_kernel"], np.float32),
        np.asarray(inputs["bias_x"], np.float32),
        np.asarray(inputs["bias_h"], np.float32),
        np.asarray(inputs["dense_w"], np.float32),
        np.asarray(inputs["dense_b"], np.float32),
    )
    wmap = {k: v.astype(np.float32 if k in _F32W else np.float16)
            for k, v in ws.items()}

    db = float(np.asarray(inputs["dense_b"], np.float64)[0])

    def one(sl):
        fx = np.zeros((n_steps, 68, NSG * CK), np.float32)
        dv = dfeat[sl, :n_steps]                     # [BS, T, F]
        dv = dv.reshape(NSG, SGC, CK, n_steps, F)
        fx[:, 0:64, :] = (dv.transpose(3, 1, 4, 0, 2)   # [T,SGC,F,NSG,CK]
                          .reshape(n_steps, 64, NSG * CK))
        yv = y0[sl, 0].reshape(NSG, SGC, CK)             # [g, ci, b]
        fx[0, 64:68, :] = (yv.transpose(1, 0, 2)
                           .reshape(4, NSG * CK) - db)
        m = {
            "feat": np.ascontiguousarray(fx).astype(np.float16),
            "h0": np.ascontiguousarray(h0[sl].T).astype(np.float16),
        }
        m.update(wmap)
        return m

    in_maps = [one(slice(i * BS, (i + 1) * BS)) for i in range(NCORES)]
    return in_maps, db


def run(inputs, trace=False, n_steps=T, **spmd_kwargs):
    """Run on the 8 NeuronCores; returns (out [B,T,1] fp32, results)."""
    from concourse.bass_utils import run_bass_kernel_spmd

    key = n_steps
    if key not in _CACHE:
        _CACHE[key] = _build_module(n_steps)
    nc = _CACHE[key]
    in_maps, db = _host_prep(inputs, n_steps)
    res = run_bass_kernel_spmd(nc, in_maps, list(range(NCORES)),
                               trace=trace, **spmd_kwargs)
    outs = np.concatenate([np.asarray(r["out"]) for r in res.results], axis=1)
    full = (outs.T[:, :, None] + np.float32(db)).astype(np.float32)
    return full, res


def kernel(**inputs) -> np.ndarray:
    out, _ = run(inputs, trace=False)
    return out


# revision 53
# speedup vs baseline: 1.6569x; 1.2780x over previous
"""GRU decoder (nn_Decoder2) Trainium2 Bass kernel, v3.14 (~256.6us; v2 290us).

Per core (pure batch data-parallel over 8 cores): 4096 rows, 8 chunks of 512,
2 supergroups (SG) of 4 chunks, partition-stacked [128, 512] so elementwise
runs 128 lanes wide.  Structure follows v2 with these changes:

  - One psum tile per (SG, gate) -- 8 tiles = 8 banks.  The Tile dep
    tracker is tile-granular, so v2's shared zr/xm tiles serialized sig r
    behind the z h-mm (+~600ns/step of false dependencies).
  - Chain-priority emission: inject/tanh/blend are emitted before the
    dense parks and t+1 feat-mms, so the scheduler (priority = emission
    order) never head-of-line-blocks the serial chain in the PE FIFO.
  - Biases moved off the feat-mms into ACT per-partition bias APs:
    sigmoid(z/r + bias_zr[128,1]), tanh(x + bias_x[128,1]); feat lhsT loses
    its ones row (K=64 for t>=1; K=68 at t=0 where rows 64:68 = y0-db
    per-chunk carry the initial y feedback for all gates).
  - Blend reform: h' = z*h + (1-z)*hh with omz = 1-z (DVE tensor_scalar,
    4x mode ~200ns) and zh = z*h prepped while inject/tanh run; after tanh
    only c = omz*hh and h' = zh + c remain on the serial chain (v2 had
    sub -> mul -> add, one more chain link).  GPSIMD is deliberately NOT
    used: its SBUF-port contention slows DVE ~15%.
  - Startup pack (31us -> 18us): fp16 weights ship as TWO dram blobs --
    A = step-0-critical (wf_*, wh0_*, whh_m), B = later-needed (whh_*,
    wd4; B's on-chip copy-outs run at the END of step 0 so they don't
    block step 0's DVE chain) -- then cheap on-chip copies to separate
    tiles (keeping all weight reads in one narrow SBUF range slowed every
    engine ~15%).  Biases are one fp32 blob; h0 prestacked per-SG
    [128, 512] interleaved with the slot-0 feat DMAs (SG0's inputs
    complete first); the inject uses the tiny early ident tile that also
    feeds ~3us of warm-up matmuls so the PE HAM clock gate reaches 8/8
    before step 0.
  - Tail: the last dense block's first 3 parks overlap step T-1.

Dense(1) -> next-input dependency folded into the recurrent weights
(whh_z/r = rk + dw x k0, whh_x = dw x k0, bias += db*k0), so the recurrence
never waits on the dense output.  Dense head batched per 4 steps: 8 mms park
at partition offsets 32*tau of the freed Z psum region, one DVE tensor_copy
evacuates all parks, 4 out-DMAs per SG per block.  Weights are prepared in
float64 and quantized to fp16 (PSUM accumulates fp32).

Steady state: 4.56-4.78us/step (5.38 on dense steps); the serial chain
r-h-mm -> sig r -> stt -> inject -> tanh -> c -> add plus ACT FIFO
contention between the two SGs sets the period.  ACT busy ~3.7us/step
(6 transcendentals) is the hard floor of this decomposition.
"""
import numpy as np

B, T, F, H = 32768, 48, 16, 32
NCORES = 8
BS = B // NCORES            # 4096 batch per core
CK = 512                    # chunk batch size
NSG = 2                     # supergroups
SGC = 4                     # chunks per supergroup

_CACHE = {}


def _prep_weights(kernel, recurrent_kernel, bias_x, bias_h, dense_w, dense_b):
    """Build v3.1 weight tiles in float64, return fp32 dict."""
    kd = kernel.astype(np.float64)
    rkd = recurrent_kernel.astype(np.float64)
    bxd = bias_x.astype(np.float64)
    bhd = bias_h.astype(np.float64)
    dwd = dense_w.astype(np.float64)[:, 0]          # [32]
    dbd = float(dense_b.astype(np.float64)[0])

    k0 = kd[0]                                      # [96]
    kf = kd[1:]                                     # [16, 96]
    dwk0 = np.outer(dwd, k0)                        # [32, 96]

    out = {}
    blocks = {"z": slice(0, 32), "r": slice(32, 64), "x": slice(64, 96)}
    bias1 = {"z": bxd[0:32] + bhd[0:32], "r": bxd[32:64] + bhd[32:64],
             "x": bxd[64:96]}
    for g, blk in blocks.items():
        # feat weights [68, 128]: rows 0:64 block-diag kf, rows 64:68 =
        # per-chunk k0 (y0-row weights, only streamed at t=0).
        wfr = np.zeros((68, 128), np.float64)
        whh = np.zeros((128, 128), np.float64)
        for c in range(4):
            cols = slice(32 * c, 32 * c + 32)
            wfr[16 * c:16 * c + 16, cols] = kf[:, blk]
            wfr[64 + c, cols] = k0[blk]
            rows = slice(32 * c, 32 * c + 32)
            if g == "x":
                whh[rows, cols] = dwk0[:, blk]
            else:
                whh[rows, cols] = rkd[:, blk] + dwk0[:, blk]
        out[f"wf_{g}"] = wfr
        out[f"whh_{g}"] = whh
        # per-partition bias AP [128,1]: bias1 + db*k0 (y0-rows carry y0-db).
        out[f"bias_{g}"] = np.tile(bias1[g] + dbd * k0[blk], 4).reshape(128, 1)
        if g != "x":
            wh0 = np.zeros((128, 128), np.float64)
            for c in range(4):
                wh0[32 * c:32 * c + 32, 32 * c:32 * c + 32] = rkd[:, blk]
            out[f"wh0_{g}"] = wh0
    whm = np.zeros((128, 128), np.float64)          # mh: h-only, all t
    for c in range(4):
        whm[32 * c:32 * c + 32, 32 * c:32 * c + 32] = rkd[:, 64:96]
    out["whh_m"] = whm
    wd4 = np.zeros((128, 4), np.float64)
    for c in range(4):
        wd4[32 * c:32 * c + 32, c] = dwd
    out["wd4"] = wd4
    out["bhm"] = np.tile(bhd[64:96], 4).reshape(128, 1)
    out["ident"] = np.eye(128)
    return {k: np.ascontiguousarray(v.astype(np.float32)) for k, v in out.items()}


_F32W = ("bhm", "bias_z", "bias_r", "bias_x")
# blob A = weights step 0 needs immediately; blob B = first needed ~2.5us
# into step 0 (ident/inject) or at t>=1 (whh_*) / t=4 (wd4)
_W16A = ("wf_z", "wf_r", "wf_x", "wh0_z", "wh0_r", "whh_m")
_W16B = ("whh_z", "whh_r", "whh_x")


def _build_module(n_steps=T):
    import concourse.bacc as bacc
    import concourse.mybir as mybir
    import concourse.tile as tile
    from contextlib import ExitStack

    f32 = mybir.dt.float32
    f16 = mybir.dt.float16
    AF = mybir.ActivationFunctionType
    ALU = mybir.AluOpType

    nc = bacc.Bacc("TRN2")
    # feat_packed [T, 68, 1024] fp16: rows 0:64 = chunk-stacked feats,
    # rows 64:68 = per-chunk (y0 - dense_b) at t=0, unused later.
    feat = nc.dram_tensor("feat", [n_steps, 68, NSG * CK], f16,
                          kind="ExternalInput")
    # startup-packed inputs: one fp16 weight blob, one fp32 bias blob,
    # per-SG prestacked h0, per-slot block-0 feats -- few large contiguous
    # DMAs instead of ~25 small ones (startup was DMA-latency-bound).
    identw = nc.dram_tensor("identw", [128, 128], f16, kind="ExternalInput")
    wblobA = nc.dram_tensor("wblobA", [128, len(_W16A) * 128], f16,
                            kind="ExternalInput")
    wblobB = nc.dram_tensor("wblobB", [128, len(_W16B) * 128 + 4], f16,
                            kind="ExternalInput")
    wblob32 = nc.dram_tensor("wblob32", [128, 4], f32, kind="ExternalInput")
    h0p = nc.dram_tensor("h0p", [NSG, 128, CK], f16, kind="ExternalInput")
    n0 = min(4, n_steps)
    feat0p = nc.dram_tensor("feat0p", [NSG, n0, 68, CK], f16,
                            kind="ExternalInput")
    out = nc.dram_tensor("out", [n_steps, BS], f32, kind="ExternalOutput")

    NBLK = (n_steps + 3) // 4

    with tile.TileContext(nc) as tc, ExitStack() as ctx:
        wpool = ctx.enter_context(tc.tile_pool(name="weights", bufs=1))
        xpool = ctx.enter_context(tc.tile_pool(name="featp", bufs=4))
        hpool = ctx.enter_context(tc.tile_pool(name="hs", bufs=1))
        ew = ctx.enter_context(tc.tile_pool(name="ew", bufs=4))
        dpool = ctx.enter_context(tc.tile_pool(name="dsb", bufs=3))
        ppool = ctx.enter_context(tc.tile_pool(name="psum", bufs=1,
                                               space="PSUM"))

        # tiny ident DMA first: the PE warm-up only needs it, so warm-up
        # starts ~6us before the main weight blobs land
        idt = wpool.tile([128, 128], f16, tag="identw")
        nc.sync.dma_start(idt[:, :], identw[:, :])
        wtA = wpool.tile([128, len(_W16A) * 128], f16, tag="wblobA")
        btile = wpool.tile([128, 4], f32, tag="wblob32")
        nc.sync.dma_start(wtA[:, :], wblobA[:, :])
        nc.sync.dma_start(btile[:, :], wblob32[:, :])
        # copy weights out of the blobs into separate SBUF tiles: the blobs
        # give few fast startup DMAs, the spread tiles avoid the uniform
        # ~15% engine slowdown seen with all weight reads hitting one
        # narrow SBUF range.
        ws = {}
        for i, n in enumerate(_W16A):
            nr = 68 if n.startswith("wf_") else 128
            t_ = wpool.tile([nr, 128], f16, tag=n)
            nc.vector.tensor_copy(t_[:, :], wtA[0:nr, 128 * i:128 * i + 128])
            ws[n] = t_
        for i, n in enumerate(_F32W):
            t_ = wpool.tile([128, 1], f32, tag=n)
            nc.vector.tensor_copy(t_[:, :], btile[:, i:i + 1])
            ws[n] = t_

        # 5x-buffered stacked state tiles per SG [128, 512] fp16:
        # blend(t) writes buf[t%5]; h-mms(t) read buf[(t+4)%5]; the dense
        # head reads states up to 4 steps late. h0 preloaded to buf 4.
        h_s = [[hpool.tile([128, CK], f16, tag=f"hs{g}_{p}", name=f"hs{g}_{p}")
                for p in range(5)] for g in range(NSG)]

        # one psum tile per (SG, gate) so the dep tracker (tile-granular)
        # doesn't serialize sig r behind the z h-mm etc.  8 tiles = 8 banks.
        gate_ps = {(g, w): ppool.tile([128, CK], f32, tag=f"ps_{w}{g}",
                                      name=f"ps_{w}{g}")
                   for g in range(NSG) for w in ("z", "r", "x", "m")}

        def bank(g, which):
            return gate_ps[(g, which)][:, :]

        # Feat block tiles [68, 2048] per SG (block b = steps 4b..4b+3).
        # Block 0 is four per-slot tiles loaded with cheap contiguous DMAs
        # (slot 0 first) so step 0 isn't gated on the full 2-block prefetch.
        feat_t = [[None] * NBLK for _ in range(NSG)]
        feat0_t = [[None] * 4 for _ in range(NSG)]

        def load_block(b):
            t0 = 4 * b
            ns = min(t0 + 4, n_steps) - t0
            for g in range(NSG):
                t_ = xpool.tile([68, 4 * CK], f16, tag=f"feat{g}",
                                name=f"feat{g}_b{b}")
                nc.sync.dma_start(
                    t_[0:64, 0:ns * CK].rearrange("r (s b) -> r s b", s=ns),
                    feat[t0:t0 + ns, 0:64, g * CK:(g + 1) * CK].rearrange(
                        "s r b -> r s b"))
                feat_t[g][b] = t_

        for ss in range(n0):
            for g in range(NSG):
                # interleave h0 with slot-0 feats so SG0's step-0 inputs
                # complete before SG1's begin (SG1 naturally lags anyway)
                if ss == 0:
                    nc.sync.dma_start(h_s[g][4][:, :], h0p[g, :, :])
                t_ = xpool.tile([68, CK], f16, tag=f"feat0s{ss}_{g}",
                                name=f"feat0s{ss}_{g}")
                nc.sync.dma_start(t_[:, :], feat0p[g, ss, :, :])
                feat0_t[g][ss] = t_
            if ss == 0:
                # PE warm-up while h0/feat DMAs land: ~3us of dummy
                # matmuls (gated only on the tiny ident DMA) flips the HAM
                # clock gate to 8/8 before step 0's real matmuls.  The
                # first start=True matmul into each bank wipes the junk.
                for i in range(26):
                    nc.tensor.matmul(bank(i % 2, "x")[0:128, 0:128],
                                     lhsT=idt[:, :], rhs=idt[:, :],
                                     start=True, stop=True,
                                     tile_position=(0, 0))
                # blob B (whh_* first needed at t=1, wd4 at t=4) queued
                # behind the step-0-critical transfers; its copy-outs are
                # emitted at the END of step 0 so they don't sit in the
                # DVE FIFO ahead of step 0's chain ops
                wtB = wpool.tile([128, len(_W16B) * 128 + 4], f16,
                                 tag="wblobB")
                nc.sync.dma_start(wtB[:, :], wblobB[:, :])

        def copy_blob_b():
            for i, n in enumerate(_W16B):
                t_ = wpool.tile([128, 128], f16, tag=n)
                nc.vector.tensor_copy(t_[:, :],
                                      wtB[:, 128 * i:128 * i + 128])
                ws[n] = t_
            t_ = wpool.tile([128, 4], f16, tag="wd4")
            nc.vector.tensor_copy(
                t_[:, :], wtB[:, len(_W16B) * 128:len(_W16B) * 128 + 4])
            ws["wd4"] = t_
        if NBLK > 1:
            load_block(1)

        def feat_rhs(g, tt, kk):
            bb, ss = tt // 4, tt % 4
            if bb == 0:
                return feat0_t[g][ss][0:kk, :]
            return feat_t[g][bb][0:kk, ss * CK:(ss + 1) * CK]

        def emit_feat(tt, gates):
            kk = 68 if tt == 0 else 64
            for gate in gates:
                for g in range(NSG):
                    nc.tensor.matmul(bank(g, gate),
                                     lhsT=ws[f"wf_{gate}"][0:kk, :],
                                     rhs=feat_rhs(g, tt, kk),
                                     start=True, stop=False,
                                     tile_position=(0, 0))

        def emit_dense_block(t0, nt):
            """Dense mms for steps t0..t0+nt-1, parked at partition offset
            32*tau of the Z psum region, one DVE evac, per-step out-DMAs.
            Emitted during step t0+4 after sig z (WAR) and before the
            z-feat wipe for t0+5."""
            for g in range(NSG):
                for tau in range(nt):
                    p0 = 32 * tau
                    nc.tensor.matmul(gate_ps[(g, "z")][p0:p0 + 4, :],
                                     lhsT=ws["wd4"][:, :],
                                     rhs=h_s[g][(t0 + tau) % 5][:, :],
                                     start=True, stop=True,
                                     tile_position=(0, p0))
            for g in range(NSG):
                npp = 32 * (nt - 1) + 4
                dsb = dpool.tile([100, CK], f32, tag=f"dsb{g}",
                                 name=f"dsb{g}_{t0}")
                # evac stays on DVE: running it on ACT shaves the dense
                # step but delays the NEXT step's sig r behind it (net loss)
                nc.vector.tensor_copy(dsb[0:npp, :],
                                      gate_ps[(g, "z")][0:npp, :])
                gb = g * SGC * CK
                for tau in range(nt):
                    nc.sync.dma_start(
                        out[t0 + tau, gb:gb + 4 * CK].rearrange(
                            "(c b) -> c b", c=4),
                        dsb[32 * tau:32 * tau + 4, :])

        for t in range(n_steps):
            blk, slot = t // 4, t % 4
            hb_in = [h_s[g][(t + 4) % 5] for g in range(NSG)]

            if t == 0:
                emit_feat(0, ("r", "z", "x"))

            # -- PE: h-mms, gate-paired across SGs; r first.  The x
            # feat-mm is emitted HERE (not in the previous step's bulk):
            # it only becomes ready at tanh(t-1), and placing it ahead of
            # r-h in the PE FIFO stalled the chain ~300ns/step.
            for gate in ("r", "z", "m"):
                wn = ("whh_m" if gate == "m" else
                      f"wh0_{gate}" if t == 0 else f"whh_{gate}")
                for g in range(NSG):
                    nc.tensor.matmul(bank(g, gate), lhsT=ws[wn][:, :],
                                     rhs=hb_in[g][:, :],
                                     start=(gate == "m"), stop=True,
                                     tile_position=(0, 0))
            if t > 0:
                emit_feat(t, ("x",))
                for g in range(NSG):
                    nc.tensor.matmul(bank(g, "x"), lhsT=ws["whh_x"][:, :],
                                     rhs=hb_in[g][:, :],
                                     start=False, stop=False,
                                     tile_position=(0, 0))

            ewt = {}
            for g in range(NSG):
                ewt[g] = {
                    k: ew.tile([128, CK], f16, tag=f"{k}{g}",
                               name=f"{k}{g}_{t}")
                    for k in ("rs", "zs", "t2", "hh", "omz", "zh", "c")
                }
            # -- serial chain: sig r -> t2 (stt reads m-psum at 1x; any
            # pre-evacuation costs MORE because every psum-source DVE op
            # is 1x-rate ~750ns -- measured, don't revisit) -------------
            for g in range(NSG):
                nc.scalar.activation(ewt[g]["rs"][:, :], bank(g, "r"),
                                     AF.Sigmoid, bias=ws["bias_r"][:, 0:1])
            for g in range(NSG):
                nc.vector.scalar_tensor_tensor(
                    ewt[g]["t2"][:, :], bank(g, "m"), ws["bhm"][:, 0:1],
                    ewt[g]["rs"][:, :], ALU.add, ALU.mult)
            for g in range(NSG):
                nc.scalar.activation(ewt[g]["zs"][:, :], bank(g, "z"),
                                     AF.Sigmoid, bias=ws["bias_z"][:, 0:1])

            # -- PE: t2-inject closes the x bank; tanh reads psum ------
            # (priority ahead of the dense parks and t+1 feats: the serial
            # chain runs through inject -> tanh -> c -> add)
            for g in range(NSG):
                nc.tensor.matmul(bank(g, "x"), lhsT=idt[:, :],
                                 rhs=ewt[g]["t2"][:, :], start=False,
                                 stop=True, tile_position=(0, 0))
            # off-chain blend prep while inject/tanh run
            for g in range(NSG):
                nc.vector.tensor_scalar(ewt[g]["omz"][:, :],
                                        ewt[g]["zs"][:, :],
                                        -1.0, 1.0, ALU.mult, ALU.add)
            for g in range(NSG):
                nc.vector.tensor_mul(ewt[g]["zh"][:, :], ewt[g]["zs"][:, :],
                                     hb_in[g][:, :])
            for g in range(NSG):
                nc.scalar.activation(ewt[g]["hh"][:, :], bank(g, "x"),
                                     AF.Tanh, bias=ws["bias_x"][:, 0:1])

            # -- chain tail: c = omz*hh, h' = zh + c -------------------
            for g in range(NSG):
                nc.vector.tensor_mul(ewt[g]["c"][:, :], ewt[g]["omz"][:, :],
                                     ewt[g]["hh"][:, :])
            for g in range(NSG):
                nc.vector.tensor_add(h_s[g][t % 5][:, :], ewt[g]["zh"][:, :],
                                     ewt[g]["c"][:, :])

            # -- PE bulk work, off the serial chain --------------------
            if t + 1 < n_steps:
                emit_feat(t + 1, ("r",))
            # dense block parks into the Z region after sig z (WAR) and
            # before the z-feat wipe for t+1
            if t > 0 and t % 4 == 0:
                emit_dense_block(t - 4, 4)
            if t == n_steps - 1 and n_steps - 1 > (n_steps - 1) // 4 * 4:
                # tail: overlap the last block's first parks with step T-1
                emit_dense_block((n_steps - 1) // 4 * 4,
                                 n_steps - 1 - (n_steps - 1) // 4 * 4)
            if t + 1 < n_steps:
                emit_feat(t + 1, ("z",))

            # -- prefetch feat block -----------------------------------
            if slot == 3 and blk + 2 < NBLK:
                load_block(blk + 2)
            if t == 0:
                copy_blob_b()

        emit_dense_block(n_steps - 1, 1)
    nc.compile()
    return nc


def _host_prep(inputs, n_steps=T):
    """Shard + pack inputs host-side. Returns (in_maps, dense_b)."""
    dfeat = np.asarray(inputs["decoder_feature"], np.float32)
    y0 = np.asarray(inputs["decoder_init_input"], np.float32)
    h0 = np.asarray(inputs["init_state"], np.float32)
    ws = _prep_weights(
        np.asarray(inputs["kernel"], np.float32),
        np.asarray(inputs["recurrent_kernel"], np.float32),
        np.asarray(inputs["bias_x"], np.float32),
        np.asarray(inputs["bias_h"], np.float32),
        np.asarray(inputs["dense_w"], np.float32),
        np.asarray(inputs["dense_b"], np.float32),
    )
    blobA = np.zeros((128, len(_W16A) * 128), np.float32)
    for i, n in enumerate(_W16A):
        w = ws[n]
        blobA[0:w.shape[0], 128 * i:128 * i + 128] = w
    blobA = np.ascontiguousarray(blobA).astype(np.float16)
    blobB = np.zeros((128, len(_W16B) * 128 + 4), np.float32)
    for i, n in enumerate(_W16B):
        blobB[:, 128 * i:128 * i + 128] = ws[n]
    blobB[:, len(_W16B) * 128:] = ws["wd4"]
    blobB = np.ascontiguousarray(blobB).astype(np.float16)
    blob32 = np.ascontiguousarray(
        np.concatenate([ws[n] for n in _F32W], axis=1)).astype(np.float32)

    db = float(np.asarray(inputs["dense_b"], np.float64)[0])
    n0 = min(4, n_steps)

    def one(sl):
        fx = np.zeros((n_steps, 68, NSG * CK), np.float32)
        dv = dfeat[sl, :n_steps]                     # [BS, T, F]
        dv = dv.reshape(NSG, SGC, CK, n_steps, F)
        fx[:, 0:64, :] = (dv.transpose(3, 1, 4, 0, 2)   # [T,SGC,F,NSG,CK]
                          .reshape(n_steps, 64, NSG * CK))
        yv = y0[sl, 0].reshape(NSG, SGC, CK)             # [g, ci, b]
        fx[0, 64:68, :] = (yv.transpose(1, 0, 2)
                           .reshape(4, NSG * CK) - db)
        fx = fx.astype(np.float16)
        h0t = h0[sl].T.astype(np.float16)                # [H, BS]
        h0pk = np.stack([                                # [NSG, 128, CK]
            np.concatenate([h0t[:, (SGC * g + ci) * CK:(SGC * g + ci + 1) * CK]
                            for ci in range(SGC)], axis=0)
            for g in range(NSG)])
        f0pk = np.stack([fx[0:n0, :, g * CK:(g + 1) * CK].transpose(0, 1, 2)
                         for g in range(NSG)])           # [NSG, n0, 68, CK]
        m = {
            "feat": np.ascontiguousarray(fx),
            "identw": np.ascontiguousarray(np.eye(128, dtype=np.float16)),
            "wblobA": blobA,
            "wblobB": blobB,
            "wblob32": blob32,
            "h0p": np.ascontiguousarray(h0pk),
            "feat0p": np.ascontiguousarray(f0pk),
        }
        return m

    in_maps = [one(slice(i * BS, (i + 1) * BS)) for i in range(NCORES)]
    return in_maps, db


def run(inputs, trace=False, n_steps=T, **spmd_kwargs):
    """Run on the 8 NeuronCores; returns (out [B,T,1] fp32, results)."""
    from concourse.bass_utils import run_bass_kernel_spmd

    key = n_steps
    if key not in _CACHE:
        _CACHE[key] = _build_module(n_steps)
    nc = _CACHE[key]
    in_maps, db = _host_prep(inputs, n_steps)
    res = run_bass_kernel_spmd(nc, in_maps, list(range(NCORES)),
                               trace=trace, **spmd_kwargs)
    outs = np.concatenate([np.asarray(r["out"]) for r in res.results], axis=1)
    full = (outs.T[:, :, None] + np.float32(db)).astype(np.float32)
    return full, res


def kernel(**inputs) -> np.ndarray:
    out, _ = run(inputs, trace=False)
    return out


# revision 55
# speedup vs baseline: 1.6642x; 1.0044x over previous
"""GRU decoder (nn_Decoder2) Trainium2 Bass kernel, v3.14 (~256.6us; v2 290us).

Per core (pure batch data-parallel over 8 cores): 4096 rows, 8 chunks of 512,
2 supergroups (SG) of 4 chunks, partition-stacked [128, 512] so elementwise
runs 128 lanes wide.  Structure follows v2 with these changes:

  - One psum tile per (SG, gate) -- 8 tiles = 8 banks.  The Tile dep
    tracker is tile-granular, so v2's shared zr/xm tiles serialized sig r
    behind the z h-mm (+~600ns/step of false dependencies).
  - Chain-priority emission: inject/tanh/blend are emitted before the
    dense parks and t+1 feat-mms, so the scheduler (priority = emission
    order) never head-of-line-blocks the serial chain in the PE FIFO.
  - Biases moved off the feat-mms into ACT per-partition bias APs:
    sigmoid(z/r + bias_zr[128,1]), tanh(x + bias_x[128,1]); feat lhsT loses
    its ones row (K=64 for t>=1; K=68 at t=0 where rows 64:68 = y0-db
    per-chunk carry the initial y feedback for all gates).
  - Blend reform: h' = z*h + (1-z)*hh with omz = 1-z (DVE tensor_scalar,
    4x mode ~200ns) and zh = z*h prepped while inject/tanh run; after tanh
    only c = omz*hh and h' = zh + c remain on the serial chain (v2 had
    sub -> mul -> add, one more chain link).  GPSIMD is deliberately NOT
    used: its SBUF-port contention slows DVE ~15%.
  - Startup pack (31us -> 18us): fp16 weights ship as TWO dram blobs --
    A = step-0-critical (wf_*, wh0_*, whh_m), B = later-needed (whh_*,
    wd4; B's on-chip copy-outs run at the END of step 0 so they don't
    block step 0's DVE chain) -- then cheap on-chip copies to separate
    tiles (keeping all weight reads in one narrow SBUF range slowed every
    engine ~15%).  Biases are one fp32 blob; h0 prestacked per-SG
    [128, 512] interleaved with the slot-0 feat DMAs (SG0's inputs
    complete first); the inject uses the tiny early ident tile that also
    feeds ~3us of warm-up matmuls so the PE HAM clock gate reaches 8/8
    before step 0.
  - Tail: the last dense block's first 3 parks overlap step T-1.

Dense(1) -> next-input dependency folded into the recurrent weights
(whh_z/r = rk + dw x k0, whh_x = dw x k0, bias += db*k0), so the recurrence
never waits on the dense output.  Dense head batched per 4 steps: 8 mms park
at partition offsets 32*tau of the freed Z psum region, one DVE tensor_copy
evacuates all parks, 4 out-DMAs per SG per block.  Weights are prepared in
float64 and quantized to fp16 (PSUM accumulates fp32).

Steady state: 4.56-4.78us/step (5.38 on dense steps); the serial chain
r-h-mm -> sig r -> stt -> inject -> tanh -> c -> add plus ACT FIFO
contention between the two SGs sets the period.  ACT busy ~3.7us/step
(6 transcendentals) is the hard floor of this decomposition.
"""
import numpy as np

B, T, F, H = 32768, 48, 16, 32
NCORES = 8
BS = B // NCORES            # 4096 batch per core
CK = 512                    # chunk batch size
NSG = 2                     # supergroups
SGC = 4                     # chunks per supergroup

_CACHE = {}


def _prep_weights(kernel, recurrent_kernel, bias_x, bias_h, dense_w, dense_b):
    """Build v3.1 weight tiles in float64, return fp32 dict."""
    kd = kernel.astype(np.float64)
    rkd = recurrent_kernel.astype(np.float64)
    bxd = bias_x.astype(np.float64)
    bhd = bias_h.astype(np.float64)
    dwd = dense_w.astype(np.float64)[:, 0]          # [32]
    dbd = float(dense_b.astype(np.float64)[0])

    k0 = kd[0]                                      # [96]
    kf = kd[1:]                                     # [16, 96]
    dwk0 = np.outer(dwd, k0)                        # [32, 96]

    out = {}
    blocks = {"z": slice(0, 32), "r": slice(32, 64), "x": slice(64, 96)}
    bias1 = {"z": bxd[0:32] + bhd[0:32], "r": bxd[32:64] + bhd[32:64],
             "x": bxd[64:96]}
    for g, blk in blocks.items():
        # feat weights [68, 128]: rows 0:64 block-diag kf, rows 64:68 =
        # per-chunk k0 (y0-row weights, only streamed at t=0).
        wfr = np.zeros((68, 128), np.float64)
        whh = np.zeros((128, 128), np.float64)
        for c in range(4):
            cols = slice(32 * c, 32 * c + 32)
            wfr[16 * c:16 * c + 16, cols] = kf[:, blk]
            wfr[64 + c, cols] = k0[blk]
            rows = slice(32 * c, 32 * c + 32)
            if g == "x":
                whh[rows, cols] = dwk0[:, blk]
            else:
                whh[rows, cols] = rkd[:, blk] + dwk0[:, blk]
        out[f"wf_{g}"] = wfr
        out[f"whh_{g}"] = whh
        # per-partition bias AP [128,1]: bias1 + db*k0 (y0-rows carry y0-db).
        out[f"bias_{g}"] = np.tile(bias1[g] + dbd * k0[blk], 4).reshape(128, 1)
        if g != "x":
            wh0 = np.zeros((128, 128), np.float64)
            for c in range(4):
                wh0[32 * c:32 * c + 32, 32 * c:32 * c + 32] = rkd[:, blk]
            out[f"wh0_{g}"] = wh0
    whm = np.zeros((128, 128), np.float64)          # mh: h-only, all t
    for c in range(4):
        whm[32 * c:32 * c + 32, 32 * c:32 * c + 32] = rkd[:, 64:96]
    out["whh_m"] = whm
    wd4 = np.zeros((128, 4), np.float64)
    for c in range(4):
        wd4[32 * c:32 * c + 32, c] = dwd
    out["wd4"] = wd4
    out["bhm"] = np.tile(bhd[64:96], 4).reshape(128, 1)
    out["ident"] = np.eye(128)
    return {k: np.ascontiguousarray(v.astype(np.float32)) for k, v in out.items()}


_F32W = ("bhm", "bias_z", "bias_r", "bias_x")
# blob A = weights step 0 needs immediately; blob B = first needed ~2.5us
# into step 0 (ident/inject) or at t>=1 (whh_*) / t=4 (wd4)
_W16A = ("wf_z", "wf_r", "wf_x", "wh0_z", "wh0_r", "whh_m")
_W16B = ("whh_z", "whh_r", "whh_x")


def _build_module(n_steps=T):
    import concourse.bacc as bacc
    import concourse.mybir as mybir
    import concourse.tile as tile
    from contextlib import ExitStack

    f32 = mybir.dt.float32
    f16 = mybir.dt.float16
    AF = mybir.ActivationFunctionType
    ALU = mybir.AluOpType

    nc = bacc.Bacc("TRN2")
    # feat_packed [T, 68, 1024] fp16: rows 0:64 = chunk-stacked feats,
    # rows 64:68 = per-chunk (y0 - dense_b) at t=0, unused later.
    feat = nc.dram_tensor("feat", [n_steps, 68, NSG * CK], f16,
                          kind="ExternalInput")
    # startup-packed inputs: one fp16 weight blob, one fp32 bias blob,
    # per-SG prestacked h0, per-slot block-0 feats -- few large contiguous
    # DMAs instead of ~25 small ones (startup was DMA-latency-bound).
    identw = nc.dram_tensor("identw", [128, 128], f16, kind="ExternalInput")
    wblobA = nc.dram_tensor("wblobA", [128, len(_W16A) * 128], f16,
                            kind="ExternalInput")
    wblobB = nc.dram_tensor("wblobB", [128, len(_W16B) * 128 + 4], f16,
                            kind="ExternalInput")
    wblob32 = nc.dram_tensor("wblob32", [128, 4], f32, kind="ExternalInput")
    h0p = nc.dram_tensor("h0p", [NSG, 128, CK], f16, kind="ExternalInput")
    n0 = min(4, n_steps)
    feat0p = nc.dram_tensor("feat0p", [NSG, n0, 68, CK], f16,
                            kind="ExternalInput")
    out = nc.dram_tensor("out", [n_steps, BS], f32, kind="ExternalOutput")

    NBLK = (n_steps + 3) // 4

    with tile.TileContext(nc) as tc, ExitStack() as ctx:
        wpool = ctx.enter_context(tc.tile_pool(name="weights", bufs=1))
        xpool = ctx.enter_context(tc.tile_pool(name="featp", bufs=4))
        hpool = ctx.enter_context(tc.tile_pool(name="hs", bufs=1))
        ew = ctx.enter_context(tc.tile_pool(name="ew", bufs=4))
        dpool = ctx.enter_context(tc.tile_pool(name="dsb", bufs=3))
        ppool = ctx.enter_context(tc.tile_pool(name="psum", bufs=1,
                                               space="PSUM"))

        # tiny ident DMA first: the PE warm-up only needs it, so warm-up
        # starts ~6us before the main weight blobs land
        idt = wpool.tile([128, 128], f16, tag="identw")
        nc.sync.dma_start(idt[:, :], identw[:, :])
        wtA = wpool.tile([128, len(_W16A) * 128], f16, tag="wblobA")
        btile = wpool.tile([128, 4], f32, tag="wblob32")
        nc.sync.dma_start(wtA[:, :], wblobA[:, :])
        nc.sync.dma_start(btile[:, :], wblob32[:, :])
        # copy weights out of the blobs into separate SBUF tiles: the blobs
        # give few fast startup DMAs, the spread tiles avoid the uniform
        # ~15% engine slowdown seen with all weight reads hitting one
        # narrow SBUF range.
        ws = {}
        for i, n in enumerate(_W16A):
            nr = 68 if n.startswith("wf_") else 128
            t_ = wpool.tile([nr, 128], f16, tag=n)
            nc.vector.tensor_copy(t_[:, :], wtA[0:nr, 128 * i:128 * i + 128])
            ws[n] = t_
        for i, n in enumerate(_F32W):
            t_ = wpool.tile([128, 1], f32, tag=n)
            nc.vector.tensor_copy(t_[:, :], btile[:, i:i + 1])
            ws[n] = t_

        # 5x-buffered stacked state tiles per SG [128, 512] fp16:
        # blend(t) writes buf[t%5]; h-mms(t) read buf[(t+4)%5]; the dense
        # head reads states up to 4 steps late. h0 preloaded to buf 4.
        h_s = [[hpool.tile([128, CK], f16, tag=f"hs{g}_{p}", name=f"hs{g}_{p}")
                for p in range(5)] for g in range(NSG)]

        # one psum tile per (SG, gate) so the dep tracker (tile-granular)
        # doesn't serialize sig r behind the z h-mm etc.  8 tiles = 8 banks.
        gate_ps = {(g, w): ppool.tile([128, CK], f32, tag=f"ps_{w}{g}",
                                      name=f"ps_{w}{g}")
                   for g in range(NSG) for w in ("z", "r", "x", "m")}

        def bank(g, which):
            return gate_ps[(g, which)][:, :]

        # Feat block tiles [68, 2048] per SG (block b = steps 4b..4b+3).
        # Block 0 is four per-slot tiles loaded with cheap contiguous DMAs
        # (slot 0 first) so step 0 isn't gated on the full 2-block prefetch.
        feat_t = [[None] * NBLK for _ in range(NSG)]
        feat0_t = [[None] * 4 for _ in range(NSG)]

        def load_block(b):
            t0 = 4 * b
            ns = min(t0 + 4, n_steps) - t0
            for g in range(NSG):
                t_ = xpool.tile([68, 4 * CK], f16, tag=f"feat{g}",
                                name=f"feat{g}_b{b}")
                nc.sync.dma_start(
                    t_[0:64, 0:ns * CK].rearrange("r (s b) -> r s b", s=ns),
                    feat[t0:t0 + ns, 0:64, g * CK:(g + 1) * CK].rearrange(
                        "s r b -> r s b"))
                feat_t[g][b] = t_

        for ss in range(n0):
            for g in range(NSG):
                # interleave h0 with slot-0 feats so SG0's step-0 inputs
                # complete before SG1's begin (SG1 naturally lags anyway)
                if ss == 0:
                    nc.sync.dma_start(h_s[g][4][:, :], h0p[g, :, :])
                t_ = xpool.tile([68, CK], f16, tag=f"feat0s{ss}_{g}",
                                name=f"feat0s{ss}_{g}")
                nc.sync.dma_start(t_[:, :], feat0p[g, ss, :, :])
                feat0_t[g][ss] = t_
            if ss == 0:
                # PE warm-up while h0/feat DMAs land: ~3us of dummy
                # matmuls (gated only on the tiny ident DMA) flips the HAM
                # clock gate to 8/8 before step 0's real matmuls.  The
                # first start=True matmul into each bank wipes the junk.
                for i in range(26):
                    nc.tensor.matmul(bank(i % 2, "x")[0:128, 0:128],
                                     lhsT=idt[:, :], rhs=idt[:, :],
                                     start=True, stop=True,
                                     tile_position=(0, 0))
                # blob B (whh_* first needed at t=1, wd4 at t=4) queued
                # behind the step-0-critical transfers; its copy-outs are
                # emitted at the END of step 0 so they don't sit in the
                # DVE FIFO ahead of step 0's chain ops
                wtB = wpool.tile([128, len(_W16B) * 128 + 4], f16,
                                 tag="wblobB")
                nc.sync.dma_start(wtB[:, :], wblobB[:, :])

        def copy_blob_b():
            for i, n in enumerate(_W16B):
                t_ = wpool.tile([128, 128], f16, tag=n)
                nc.vector.tensor_copy(t_[:, :],
                                      wtB[:, 128 * i:128 * i + 128])
                ws[n] = t_
            t_ = wpool.tile([128, 4], f16, tag="wd4")
            nc.vector.tensor_copy(
                t_[:, :], wtB[:, len(_W16B) * 128:len(_W16B) * 128 + 4])
            ws["wd4"] = t_
        if NBLK > 1:
            load_block(1)

        def feat_rhs(g, tt, kk):
            bb, ss = tt // 4, tt % 4
            if bb == 0:
                return feat0_t[g][ss][0:kk, :]
            return feat_t[g][bb][0:kk, ss * CK:(ss + 1) * CK]

        def emit_feat(tt, gates):
            kk = 68 if tt == 0 else 64
            for gate in gates:
                for g in range(NSG):
                    nc.tensor.matmul(bank(g, gate),
                                     lhsT=ws[f"wf_{gate}"][0:kk, :],
                                     rhs=feat_rhs(g, tt, kk),
                                     start=True, stop=False,
                                     tile_position=(0, 0))

        def emit_dense_block(t0, nt):
            """Dense mms for steps t0..t0+nt-1, parked at partition offset
            32*tau of the Z psum region, one DVE evac, per-step out-DMAs.
            Emitted during step t0+4 after sig z (WAR) and before the
            z-feat wipe for t0+5."""
            for g in range(NSG):
                for tau in range(nt):
                    p0 = 32 * tau
                    nc.tensor.matmul(gate_ps[(g, "z")][p0:p0 + 4, :],
                                     lhsT=ws["wd4"][:, :],
                                     rhs=h_s[g][(t0 + tau) % 5][:, :],
                                     start=True, stop=True,
                                     tile_position=(0, p0))
            for g in range(NSG):
                npp = 32 * (nt - 1) + 4
                dsb = dpool.tile([100, CK], f32, tag=f"dsb{g}",
                                 name=f"dsb{g}_{t0}")
                # evac stays on DVE: running it on ACT shaves the dense
                # step but delays the NEXT step's sig r behind it (net loss)
                nc.vector.tensor_copy(dsb[0:npp, :],
                                      gate_ps[(g, "z")][0:npp, :])
                gb = g * SGC * CK
                for tau in range(nt):
                    nc.sync.dma_start(
                        out[t0 + tau, gb:gb + 4 * CK].rearrange(
                            "(c b) -> c b", c=4),
                        dsb[32 * tau:32 * tau + 4, :])

        for t in range(n_steps):
            blk, slot = t // 4, t % 4
            hb_in = [h_s[g][(t + 4) % 5] for g in range(NSG)]

            # ALL feat-mms for step t are emitted in step t's own body
            # (ahead of the matching h-mm for bank-group order): they're
            # ready early (WAR on step t-1's sigmoids/tanh) and fill the
            # PE's blend-wait window without queueing ahead of chain ops.
            if t == 0:
                emit_feat(0, ("r", "z", "x"))
            else:
                emit_feat(t, ("r", "z"))

            # -- PE: h-mms, gate-paired across SGs; r first -------------
            for gate in ("r", "z", "m"):
                wn = ("whh_m" if gate == "m" else
                      f"wh0_{gate}" if t == 0 else f"whh_{gate}")
                for g in range(NSG):
                    nc.tensor.matmul(bank(g, gate), lhsT=ws[wn][:, :],
                                     rhs=hb_in[g][:, :],
                                     start=(gate == "m"), stop=True,
                                     tile_position=(0, 0))
            if t > 0:
                emit_feat(t, ("x",))
                for g in range(NSG):
                    nc.tensor.matmul(bank(g, "x"), lhsT=ws["whh_x"][:, :],
                                     rhs=hb_in[g][:, :],
                                     start=False, stop=False,
                                     tile_position=(0, 0))

            ewt = {}
            for g in range(NSG):
                ewt[g] = {
                    k: ew.tile([128, CK], f16, tag=f"{k}{g}",
                               name=f"{k}{g}_{t}")
                    for k in ("rs", "zs", "t2", "hh", "omz", "zh", "c")
                }
            # -- serial chain: sig r -> t2 (stt reads m-psum at 1x; any
            # pre-evacuation costs MORE because every psum-source DVE op
            # is 1x-rate ~750ns -- measured, don't revisit) -------------
            for g in range(NSG):
                nc.scalar.activation(ewt[g]["rs"][:, :], bank(g, "r"),
                                     AF.Sigmoid, bias=ws["bias_r"][:, 0:1])
            for g in range(NSG):
                nc.vector.scalar_tensor_tensor(
                    ewt[g]["t2"][:, :], bank(g, "m"), ws["bhm"][:, 0:1],
                    ewt[g]["rs"][:, :], ALU.add, ALU.mult)
            for g in range(NSG):
                nc.scalar.activation(ewt[g]["zs"][:, :], bank(g, "z"),
                                     AF.Sigmoid, bias=ws["bias_z"][:, 0:1])

            # -- PE: t2-inject closes the x bank; tanh reads psum ------
            # (priority ahead of the dense parks and t+1 feats: the serial
            # chain runs through inject -> tanh -> c -> add)
            for g in range(NSG):
                nc.tensor.matmul(bank(g, "x"), lhsT=idt[:, :],
                                 rhs=ewt[g]["t2"][:, :], start=False,
                                 stop=True, tile_position=(0, 0))
            # off-chain blend prep while inject/tanh run
            for g in range(NSG):
                nc.vector.tensor_scalar(ewt[g]["omz"][:, :],
                                        ewt[g]["zs"][:, :],
                                        -1.0, 1.0, ALU.mult, ALU.add)
            for g in range(NSG):
                nc.vector.tensor_mul(ewt[g]["zh"][:, :], ewt[g]["zs"][:, :],
                                     hb_in[g][:, :])
            for g in range(NSG):
                nc.scalar.activation(ewt[g]["hh"][:, :], bank(g, "x"),
                                     AF.Tanh, bias=ws["bias_x"][:, 0:1])

            # -- chain tail: c = omz*hh, h' = zh + c -------------------
            for g in range(NSG):
                nc.vector.tensor_mul(ewt[g]["c"][:, :], ewt[g]["omz"][:, :],
                                     ewt[g]["hh"][:, :])
            for g in range(NSG):
                nc.vector.tensor_add(h_s[g][t % 5][:, :], ewt[g]["zh"][:, :],
                                     ewt[g]["c"][:, :])

            # -- PE bulk work, off the serial chain --------------------
            # dense block parks into the Z region after sig z (WAR) and
            # before the z-feat wipe for t+1
            if t > 0 and t % 4 == 0:
                emit_dense_block(t - 4, 4)
            if t == n_steps - 1 and n_steps - 1 > (n_steps - 1) // 4 * 4:
                # tail: overlap the last block's first parks with step T-1
                emit_dense_block((n_steps - 1) // 4 * 4,
                                 n_steps - 1 - (n_steps - 1) // 4 * 4)

            # -- prefetch feat block -----------------------------------
            if slot == 3 and blk + 2 < NBLK:
                load_block(blk + 2)
            if t == 0:
                copy_blob_b()

        emit_dense_block(n_steps - 1, 1)
    nc.compile()
    return nc


def _host_prep(inputs, n_steps=T):
    """Shard + pack inputs host-side. Returns (in_maps, dense_b)."""
    dfeat = np.asarray(inputs["decoder_feature"], np.float32)
    y0 = np.asarray(inputs["decoder_init_input"], np.float32)
    h0 = np.asarray(inputs["init_state"], np.float32)
    ws = _prep_weights(
        np.asarray(inputs["kernel"], np.float32),
        np.asarray(inputs["recurrent_kernel"], np.float32),
        np.asarray(inputs["bias_x"], np.float32),
        np.asarray(inputs["bias_h"], np.float32),
        np.asarray(inputs["dense_w"], np.float32),
        np.asarray(inputs["dense_b"], np.float32),
    )
    blobA = np.zeros((128, len(_W16A) * 128), np.float32)
    for i, n in enumerate(_W16A):
        w = ws[n]
        blobA[0:w.shape[0], 128 * i:128 * i + 128] = w
    blobA = np.ascontiguousarray(blobA).astype(np.float16)
    blobB = np.zeros((128, len(_W16B) * 128 + 4), np.float32)
    for i, n in enumerate(_W16B):
        blobB[:, 128 * i:128 * i + 128] = ws[n]
    blobB[:, len(_W16B) * 128:] = ws["wd4"]
    blobB = np.ascontiguousarray(blobB).astype(np.float16)
    blob32 = np.ascontiguousarray(
        np.concatenate([ws[n] for n in _F32W], axis=1)).astype(np.float32)

    db = float(np.asarray(inputs["dense_b"], np.float64)[0])
    n0 = min(4, n_steps)

    def one(sl):
        fx = np.zeros((n_steps, 68, NSG * CK), np.float32)
        dv = dfeat[sl, :n_steps]                     # [BS, T, F]
        dv = dv.reshape(NSG, SGC, CK, n_steps, F)
        fx[:, 0:64, :] = (dv.transpose(3, 1, 4, 0, 2)   # [T,SGC,F,NSG,CK]
                          .reshape(n_steps, 64, NSG * CK))
        yv = y0[sl, 0].reshape(NSG, SGC, CK)             # [g, ci, b]
        fx[0, 64:68, :] = (yv.transpose(1, 0, 2)
                           .reshape(4, NSG * CK) - db)
        fx = fx.astype(np.float16)
        h0t = h0[sl].T.astype(np.float16)                # [H, BS]
        h0pk = np.stack([                                # [NSG, 128, CK]
            np.concatenate([h0t[:, (SGC * g + ci) * CK:(SGC * g + ci + 1) * CK]
                            for ci in range(SGC)], axis=0)
            for g in range(NSG)])
        f0pk = np.stack([fx[0:n0, :, g * CK:(g + 1) * CK].transpose(0, 1, 2)
                         for g in range(NSG)])           # [NSG, n0, 68, CK]
        m = {
            "feat": np.ascontiguousarray(fx),
            "identw": np.ascontiguousarray(np.eye(128, dtype=np.float16)),
            "wblobA": blobA,
            "wblobB": blobB,
            "wblob32": blob32,
            "h0p": np.ascontiguousarray(h0pk),
            "feat0p": np.ascontiguousarray(f0pk),
        }
        return m

    in_maps = [one(slice(i * BS, (i + 1) * BS)) for i in range(NCORES)]
    return in_maps, db


def run(inputs, trace=False, n_steps=T, **spmd_kwargs):
    """Run on the 8 NeuronCores; returns (out [B,T,1] fp32, results)."""
    from concourse.bass_utils import run_bass_kernel_spmd

    key = n_steps
    if key not in _CACHE:
        _CACHE[key] = _build_module(n_steps)
    nc = _CACHE[key]
    in_maps, db = _host_prep(inputs, n_steps)
    res = run_bass_kernel_spmd(nc, in_maps, list(range(NCORES)),
                               trace=trace, **spmd_kwargs)
    outs = np.concatenate([np.asarray(r["out"]) for r in res.results], axis=1)
    full = (outs.T[:, :, None] + np.float32(db)).astype(np.float32)
    return full, res


def kernel(**inputs) -> np.ndarray:
    out, _ = run(inputs, trace=False)
    return out
